# revision 8
# baseline (speedup 1.0000x reference)
"""GroupedQueryAttention Trainium2 kernel (8-core SPMD), v6.

Reference op: RMSNorm -> in-proj (q/k/v) -> RoPE -> causal GQA attention
-> out-proj -> residual.  b=2, s=2048, d_model=2048, 32 q-heads / 8 KV
groups, head dim 64, fp32 in/out (f16 on device).

Sharding: core c handles batch b = c//4 and KV groups (2j, 2j+1), j = c%4.
Each core computes the in-projection restricted to its 8 heads' channels,
attention for its 8 heads, and a partial out-projection (row-parallel).
The host sums the 4 partials per batch and adds the residual.

v6 design (vs v2 baseline):
  * inv_rms precomputed on the host (a [B,S] reduction is negligible in
    numpy): removes all x^2 ops, 256 ss matmuls and the Newton chain.
  * causal mask folded into the PSUM scores on the PE itself: one
    id.T @ [mneg|mneg] accumulation per diagonal tile adds -6e4 above
    the diagonal for both heads, so exp yields exact zeros and no
    DVE mask sits between exp and the AV matmuls.
  * rope fully in f16 on DVE (2x mode) with single-replica cos/sin
    tables read through stride-0 mid-dim broadcast APs.
  * per-q-tile early softmax normalization: q-tile qt is normalized and
    queued for its o-pack transpose right after its last AV
    contribution (t = 4c+qt), shrinking the next pair's WAR stall on
    the shared PSUM accumulators.
  * x stored tok-tile-major in DRAM ([128, tt, kt, 128]) and loaded as
    4 per-tile DMAs; w_in preload split in kt-quarters ordered for the
    cold start; output DMA split per 4 m-slices for the tail.
  * tr-pool double buffered; qkT drains on DVE; emission weave as v2:
    P0(c) interleaves with ATT(c-1) at tok-tile granularity with
    transpose/o-pack fillers between matmul groups.
"""

import os
import numpy as np
from contextlib import ExitStack

import concourse.bass as bass
from concourse import bacc as _bacc
import concourse.mybir as mybir
import concourse.tile as tile
from concourse.bass import ts

f32 = mybir.dt.float32
f16 = mybir.dt.float16
AF = mybir.ActivationFunctionType
ALU = mybir.AluOpType

D = 2048          # model dim
CH = 768          # per-core in-proj channels (8 q heads + 2 k + 2 v)
TOKC = 512        # token chunk
NKT = D // 128    # 16 k-tiles over model dim
RMS_EPS = 1e-6
ROPE_THETA = 10000.0
NCORES = 8


def build_program(S=2048):
    NCH = S // TOKC          # token chunks
    NT = S // 128            # token/key tiles
    nc = _bacc.Bacc(None)

    xT_d = nc.dram_tensor("xT", [128, NT * NKT * 128], f16,
                          kind="ExternalInput")
    inv_d = nc.dram_tensor("invr", [128, NT], f32, kind="ExternalInput")
    w_in_d = nc.dram_tensor("w_in_p", [128, NKT * CH], f16, kind="ExternalInput")
    w_out_d = nc.dram_tensor("w_out_p", [128, 4 * D], f16, kind="ExternalInput")
    cos2_d = nc.dram_tensor("cos2", [128, NT * 64], f16, kind="ExternalInput")
    sinpm_d = nc.dram_tensor("sinpm", [128, NT * 64], f16, kind="ExternalInput")
    mneg_d = nc.dram_tensor("mneg", [128, 256], f16, kind="ExternalInput")
    id_d = nc.dram_tensor("id128", [128, 128], f16, kind="ExternalInput")
    yT_d = nc.dram_tensor("yT", [D, S], f16, kind="ExternalOutput")

    with tile.TileContext(nc) as tc, ExitStack() as ctx:
        sb = ctx.enter_context(tc.tile_pool(name="sb", bufs=1))
        sbs = ctx.enter_context(tc.tile_pool(name="sbs", bufs=2))

        # ---------------- persistent SBUF ----------------
        w_in_sb = sb.tile([128, NKT, CH], f16, name="w_in_sb")
        w_out_sb = sb.tile([128, 4, D], f16, name="w_out_sb")
        cos2_sb = sb.tile([128, NT, 64], f16, name="cos2_sb")
        sinpm_sb = sb.tile([128, NT, 64], f16, name="sinpm_sb")
        mneg_sb = sb.tile([128, 2, 128], f16, name="mneg_sb")
        id_sb = sb.tile([128, 128], f16, name="id_sb")
        ones_sb = sb.tile([128, 1], f16, name="ones_sb")
        zer_sb = sb.tile([128, 4, 65], f16, name="zer_sb")
        qkT = sb.tile([128, 5, S], f16, name="qkT")     # feat-major roped q(4)/k(1)
        vAB = sb.tile([128, NT, 2, 65], f16, name="vAB")
        oT = sb.tile([128, 4, S], f16, name="oT")       # feat-major o per pair
        inv_sb = sb.tile([128, NT], f32, name="inv_sb")  # host-computed rsqrt

        # preloads all on the scalar queue, ordered by first use; the sync
        # queue stays free so the x(0) load (emitted in the schedule) is
        # serviced immediately.
        w_in_r = w_in_d.rearrange("p (o c) -> p o c", c=CH)
        nc.scalar.dma_start(id_sb[:], id_d[:])
        nc.scalar.dma_start(inv_sb[:], inv_d[:])
        nc.scalar.dma_start(w_in_sb[:, 0:4, :], w_in_r[:, 0:4, :])
        nc.scalar.dma_start(cos2_sb[:], cos2_d.rearrange("p (o c) -> p o c", c=64))
        nc.scalar.dma_start(sinpm_sb[:],
                            sinpm_d.rearrange("p (o c) -> p o c", c=64))
        for q in range(1, 4):
            nc.scalar.dma_start(w_in_sb[:, 4 * q:4 * q + 4, :],
                                w_in_r[:, 4 * q:4 * q + 4, :])
        nc.scalar.dma_start(mneg_sb[:],
                            mneg_d.rearrange("p (h i) -> p h i", i=128))
        nc.scalar.dma_start(w_out_sb[:], w_out_d.rearrange("p (o c) -> p o c", c=D))
        nc.gpsimd.memset(ones_sb[:], 1.0)
        nc.gpsimd.memset(zer_sb[:], 0.0)
        # contiguous full-tile memset; V copies later overwrite cols 0:64
        # of each [*, t, h] slice, leaving column 64 as the ones column.
        nc.gpsimd.memset(vAB[:], 1.0)

        with tc.tile_pool(name="ps", bufs=1, space="PSUM") as ps:
            # PSUM budget (8 banks): big 2x2 + avA 1 + avB 1 + ss 1 + tr 1.

            # deferred-emission queue: thunks sprinkled between matmul
            # groups so single-buffered PSUM tags never stall the PE.
            filler_q = []

            def drain(n=1):
                for _ in range(n):
                    if filler_q:
                        filler_q.pop(0)()

            def drain_all():
                while filler_q:
                    filler_q.pop(0)()

            xchunks = {}

            def emit_load_x(c):
                # tile-major DRAM layout: per-tok-tile DMAs, 4KB runs.
                xc = sbs.tile([128, 4, NKT, 128], f16, tag="xc", bufs=2,
                              name=f"xc_{c}")
                xr = xT_d.rearrange("p (tt kt i) -> p tt kt i", kt=NKT, i=128)
                for t in range(4):
                    nc.sync.dma_start(xc[:, t], xr[:, 4 * c + t])
                xchunks[c] = xc

            def emit_P0_tau(c, t):
                """in-proj for tok-tile t of chunk c + rope chain
                (inv_rms is precomputed host-side).  Fillers drain
                between k-tile groups."""
                xc = xchunks[c]
                tg = 4 * c + t
                ip = ps.tile([128, 2, TOKC], f32, tag="big", bufs=2,
                             name=f"ip_{c}_{t}")
                for kt in range(NKT):
                    nc.tensor.matmul(ip[:, 0, 0:384], xc[:, t, kt, :],
                                     w_in_sb[:, kt, 0:384],
                                     start=(kt == 0), stop=(kt == NKT - 1))
                    nc.tensor.matmul(ip[:, 1, 0:384], xc[:, t, kt, :],
                                     w_in_sb[:, kt, 384:768],
                                     start=(kt == 0), stop=(kt == NKT - 1))
                    if kt % 5 == 4:
                        drain()
                # --- apply inv_rms to the whole q/k block in two ACT
                # copies (per-partition scale); rope then reads SBUF with
                # unscaled, preloaded tables.
                qn = sbs.tile([128, 640], f16, tag="qn", bufs=3,
                              name=f"qn_{c}_{t}")
                nc.scalar.activation(qn[:, 0:384], ip[:, 0, 0:384], AF.Copy,
                                     scale=inv_sb[:, tg:tg + 1])
                nc.scalar.activation(qn[:, 384:640], ip[:, 1, 0:256], AF.Copy,
                                     scale=inv_sb[:, tg:tg + 1])
                def bc(tab, lo, hi, h):
                    return tab[:, tg, lo:hi].unsqueeze(1).to_broadcast(
                        (128, h, hi - lo))
                # --- rope (token-major).  Block A: q heads 0-5; block B:
                # q heads 6,7 + k0,k1 (all rope identically).
                qt_sb = sbs.tile([128, 640], f16, tag="qt_sb", bufs=3,
                                 name=f"qt_{c}_{t}")
                tmpA = sbs.tile([128, 6, 64], f16, tag="tmpA", bufs=2,
                                name=f"tmpA_{c}_{t}")
                tmpB = sbs.tile([128, 4, 64], f16, tag="tmpB", bufs=2,
                                name=f"tmpB_{c}_{t}")
                cqA = sbs.tile([128, 6, 64], f16, tag="cqA", bufs=2,
                               name=f"cqA_{c}_{t}")
                cqB = sbs.tile([128, 4, 64], f16, tag="cqB", bufs=2,
                               name=f"cqB_{c}_{t}")
                blkA = qn[:, 0:384].rearrange("p (h d) -> p h d", d=64)
                blkB = qn[:, 384:640].rearrange("p (h d) -> p h d", d=64)
                nc.vector.tensor_tensor(
                    tmpA[:, :, 0:32], blkA[:, :, 32:64],
                    bc(sinpm_sb, 0, 32, 6), ALU.mult)
                nc.vector.tensor_tensor(
                    tmpA[:, :, 32:64], blkA[:, :, 0:32],
                    bc(sinpm_sb, 32, 64, 6), ALU.mult)
                nc.vector.tensor_tensor(cqA[:], blkA[:], bc(cos2_sb, 0, 64, 6),
                                        ALU.mult)
                nc.vector.tensor_tensor(
                    qt_sb[:, 0:384].rearrange("p (h d) -> p h d", d=64),
                    cqA[:], tmpA[:], ALU.add)
                nc.vector.tensor_tensor(
                    tmpB[:, :, 0:32], blkB[:, :, 32:64],
                    bc(sinpm_sb, 0, 32, 4), ALU.mult)
                nc.vector.tensor_tensor(
                    tmpB[:, :, 32:64], blkB[:, :, 0:32],
                    bc(sinpm_sb, 32, 64, 4), ALU.mult)
                nc.vector.tensor_tensor(cqB[:], blkB[:], bc(cos2_sb, 0, 64, 4),
                                        ALU.mult)
                nc.vector.tensor_tensor(
                    qt_sb[:, 384:640].rearrange("p (h d) -> p h d", d=64),
                    cqB[:], tmpB[:], ALU.add)
                # --- V: per-token inv scale during PSUM->SBUF copy
                nc.vector.tensor_tensor(
                    vAB[:, tg, :, 0:64],
                    ip[:, 1, 256:384].rearrange("p (h d) -> p h d", d=64),
                    inv_sb[:, tg:tg + 1].to_broadcast((128, 2, 64)), ALU.mult)
                # transpose q/k of this tok-tile to feature-major
                # (deferred: reads qt_sb, which the DVE/Pool chain above
                # still has to produce; spread over later mm groups).
                for ct in range(5):
                    filler_q.append(
                        lambda tg=tg, ct=ct, qt_sb=qt_sb: emit_tr(tg, ct, qt_sb))

            def emit_tr(tg, ct, qt_sb):
                trp = ps.tile([128, 128], f16, tag="tr", bufs=2,
                              name=f"tr_{tg}_{ct}")
                nc.tensor.transpose(trp[:], qt_sb[:, ts(ct, 128)], id_sb[:])
                nc.vector.tensor_copy(qkT[:, ct, ts(tg, 128)], trp[:])

            def emit_att_pair(c, p):
                """attention for q-chunk c, head-pair p (heads of groups
                g0,g1 at q ch-tile p; k ch-tile 4)."""
                n_t = 4 * (c + 1)
                # One PSUM bank supports a single accumulation group at a
                # time (2KB zero region), so the 4 concurrent per-q-tile
                # accumulators share a bank via explicit memset + pure
                # accumulation (start=False, skip_group_check).
                avA = ps.tile([128, 4, 65], f32, tag="avA", bufs=1,
                              name=f"avA_{c}_{p}")
                avB = ps.tile([128, 4, 65], f32, tag="avB", bufs=1,
                              name=f"avB_{c}_{p}")
                qks = []

                def emit_qk(t):
                    j0 = max(0, t - 4 * c) * 128
                    diag = t >= 4 * c
                    qk = ps.tile([128, 2, TOKC], f32, tag="big", bufs=2,
                                 name=f"qk_{c}_{p}_{t}")
                    for h in (0, 1):
                        nc.tensor.matmul(
                            qk[:, h, j0:], qkT[64 * h:64 * h + 64, 4, ts(t, 128)],
                            qkT[64 * h:64 * h + 64, p,
                                c * TOKC + j0:(c + 1) * TOKC],
                            start=True, stop=not diag)
                    if diag:
                        # fold the causal mask into the PSUM scores for
                        # both heads at once: += id.T @ [mneg|mneg] adds
                        # -6e4 above the diagonal; exp gives exact zeros.
                        nc.tensor.matmul(
                            qk[:, :, j0:j0 + 128], id_sb[:], mneg_sb[:],
                            start=False, stop=True, skip_group_check=True)
                    qks.append(qk)

                emit_qk(0)
                # o-pack fillers + av zero-init fill the WAR window before
                # qk(1) can reuse the previous pair's PSUM buffer.
                drain(3)
                nc.tensor.matmul(avA[:], id_sb[:], zer_sb[:],
                                 start=True, stop=True)
                nc.tensor.matmul(avB[:], id_sb[:], zer_sb[:],
                                 start=True, stop=True)
                for t in range(n_t):
                    if t + 1 < n_t:
                        emit_qk(t + 1)
                    j0 = max(0, t - 4 * c) * 128
                    qk = qks[t]
                    e = sbs.tile([128, 2, TOKC], f16, tag="e", bufs=4,
                                 name=f"e_{c}_{p}_{t}")
                    nc.scalar.activation(e[:, :, j0:], qk[:, :, j0:], AF.Exp)
                    if t < n_t - 2:
                        drain()
                    for qt in range(4):
                        if 4 * c + qt < t:
                            continue
                        nc.tensor.matmul(avA[:, qt, :],
                                         e[:, 0, ts(qt, 128)], vAB[:, t, 0, :],
                                         start=False, stop=False,
                                         skip_group_check=True)
                        nc.tensor.matmul(avB[:, qt, :],
                                         e[:, 1, ts(qt, 128)], vAB[:, t, 1, :],
                                         start=False, stop=False,
                                         skip_group_check=True)
                    # q-tile qt gets its last AV contribution at
                    # t = 4c + qt: normalize + queue its o-pack right
                    # away so the next pair's WAR on av shrinks.
                    qt = t - 4 * c
                    if qt >= 0:
                        d2 = sbs.tile([128, 2], f32, tag="d2", bufs=4,
                                      name=f"d2_{c}_{p}_{qt}")
                        nc.vector.reciprocal(d2[:, 0:1], avA[:, qt, 64:65])
                        nc.vector.reciprocal(d2[:, 1:2], avB[:, qt, 64:65])
                        pk = sbs.tile([128, 128], f16, tag="pk", bufs=3,
                                      name=f"pk_{c}_{p}_{qt}")
                        nc.vector.tensor_tensor(
                            pk[:, 0:64], avA[:, qt, 0:64],
                            d2[:, 0:1].to_broadcast((128, 64)), ALU.mult)
                        nc.vector.tensor_tensor(
                            pk[:, 64:128], avB[:, qt, 0:64],
                            d2[:, 1:2].to_broadcast((128, 64)), ALU.mult)
                        filler_q.append(
                            lambda c=c, p=p, qt=qt, pk=pk:
                            emit_opack(c, p, qt, pk))

            def emit_opack(c, p, qt, pk):
                trp = ps.tile([128, 128], f16, tag="tr", bufs=2,
                              name=f"otr_{c}_{p}_{qt}")
                nc.tensor.transpose(trp[:], pk[:], id_sb[:])
                nc.vector.tensor_copy(oT[:, p, c * TOKC + qt * 128:
                                         c * TOKC + (qt + 1) * 128], trp[:])

            def emit_out(c):
                cs = slice(c * TOKC, (c + 1) * TOKC)
                yo = sbs.tile([128, 16, TOKC], f16, tag="yo", bufs=1,
                              name=f"yo_{c}")
                for m in range(16):
                    op = ps.tile([128, 2, TOKC], f32, tag="big", bufs=2,
                                 name=f"op_{c}_{m}")
                    for kt in range(4):
                        nc.tensor.matmul(op[:, 0, :], w_out_sb[:, kt, ts(m, 128)],
                                         oT[:, kt, cs],
                                         start=(kt == 0), stop=(kt == 3))
                    nc.vector.tensor_copy(yo[:, m, :], op[:, 0, :])
                    if m % 3 == 2:
                        drain()
                    if m % 4 == 3:
                        nc.sync.dma_start(
                            yT_d.rearrange("(o p) s -> p o s", p=128)[
                                0:128, m - 3:m + 1, cs], yo[:, m - 3:m + 1, :])

            # ------------------- schedule -------------------
            # depth-1 stagger: attention for chunk c-1 weaves into P0(c).
            # The drain_all() at iteration start flushes TR(c-1) so the
            # whole qkT range for keys <= c-1 is emitted before ATT(c-1).
            # warm-up: ~3us of junk transposes ramp the PE clock to full
            # speed while the first x/w DMAs stream in.
            warm = ps.tile([128, 128], f16, tag="tr", bufs=2, name="warm")
            for _ in range(12):
                nc.tensor.transpose(warm[:], id_sb[:], id_sb[:])
            emit_load_x(0)
            for c in range(NCH + 1):
                drain_all()
                for t in range(4):
                    if t == 2 and c + 1 < NCH:
                        emit_load_x(c + 1)
                    if c < NCH:
                        emit_P0_tau(c, t)
                    if 0 <= c - 1 < NCH:
                        emit_att_pair(c - 1, t)
                if 0 <= c - 1 < NCH:
                    drain_all()   # flush o-pack of pair 3 before out-proj
                    emit_out(c - 1)
            drain_all()

    nc.finalize()
    return nc


# ------------------------------- host side ----------------------------------

def _rope_tables(S):
    NT = S // 128
    inv_freq = ROPE_THETA ** (-np.arange(0, 64, 2, dtype=np.float64) / 64.0)
    t = np.arange(S, dtype=np.float64)[:, None]            # [S, 1]
    ang = t * inv_freq[None, :]                            # [S, 32]
    cos = np.cos(ang)
    sin = np.sin(ang)
    cos2 = np.concatenate([cos, cos], axis=1)              # [S, 64]
    sinpm = np.concatenate([-sin, sin], axis=1)            # [S, 64]
    cos2 = cos2.reshape(NT, 128, 64).transpose(1, 0, 2).reshape(128, NT * 64)
    sinpm = sinpm.reshape(NT, 128, 64).transpose(1, 0, 2).reshape(128, NT * 64)
    return (np.ascontiguousarray(cos2, dtype=np.float16),
            np.ascontiguousarray(sinpm, dtype=np.float16))


def host_prepare(x, w_in, w_out, rms_w):
    S = x.shape[1]
    NT = S // 128
    x = np.asarray(x, dtype=np.float32)
    w_eff = np.asarray(w_in, dtype=np.float32) * np.asarray(rms_w, np.float32)[None, :]
    w_out = np.asarray(w_out, dtype=np.float32)
    cos2, sinpm = _rope_tables(S)
    mneg1 = np.tril(np.full((128, 128), -60000.0, dtype=np.float32), -1)
    mneg = np.ascontiguousarray(np.concatenate([mneg1, mneg1], axis=1))
    id128 = np.eye(128, dtype=np.float32)
    qscale = np.float32(64 ** -0.5)
    # host-side RMS inverse: [B, S] -> per-core [128, NT]
    inv_all = 1.0 / np.sqrt((x.astype(np.float32) ** 2).mean(-1) + RMS_EPS)

    in_maps = []
    for core in range(NCORES):
        b, j = divmod(core, 4)
        g0, g1 = 2 * j, 2 * j + 1
        rows = []
        for p in range(4):
            for g in (g0, g1):
                rows.extend(range((g * 4 + p) * 64, (g * 4 + p) * 64 + 64))
        for g in (g0, g1):
            rows.extend(range(2048 + g * 64, 2048 + g * 64 + 64))
        for g in (g0, g1):
            rows.extend(range(2560 + g * 64, 2560 + g * 64 + 64))
        w_slice = w_eff[rows, :].copy()          # [768, 2048]
        w_slice[:512, :] *= qscale
        # device layout: w_in_p[p, kt*768 + ch] = w_slice[ch, kt*128 + p]
        w_in_p = w_slice.T.reshape(NKT, 128, CH).transpose(1, 0, 2).reshape(
            128, NKT * CH)
        cols = []
        for p in range(4):
            for g in (g0, g1):
                cols.extend(range((g * 4 + p) * 64, (g * 4 + p) * 64 + 64))
        w_o = w_out[:, cols]                     # [2048, 512]
        # device layout: w_out_p[p, kt*2048 + m] = w_o[m, kt*128 + p]
        w_out_p = w_o.T.reshape(4, 128, D).transpose(1, 0, 2).reshape(128, 4 * D)
        xb = x[b].reshape(NT, 128, NKT, 128)     # [tt, i, kt, p]
        xT = xb.transpose(3, 0, 2, 1).reshape(128, NT * NKT * 128)
        invr = np.ascontiguousarray(
            inv_all[b].reshape(NT, 128).T).astype(np.float32)
        in_maps.append({
            "xT": np.ascontiguousarray(xT).astype(np.float16),
            "invr": invr,
            "w_in_p": np.ascontiguousarray(w_in_p).astype(np.float16),
            "w_out_p": np.ascontiguousarray(w_out_p).astype(np.float16),
            "cos2": cos2.astype(np.float16),
            "sinpm": sinpm.astype(np.float16),
            "mneg": mneg.astype(np.float16),
            "id128": id128.astype(np.float16),
        })
    return in_maps


def assemble(x, results):
    x = np.asarray(x, dtype=np.float32)
    out = np.empty_like(x)
    for b in range(2):
        acc = np.zeros((D, x.shape[1]), dtype=np.float32)
        for j in range(4):
            acc += results[4 * b + j]["yT"].astype(np.float32)
        out[b] = x[b] + acc.T
    return out


_PROGRAMS = {}


def _get_program(S):
    if S not in _PROGRAMS:
        _PROGRAMS[S] = build_program(S)
    return _PROGRAMS[S]


def run(x, w_in, w_out, rms_w, trace=False):
    from concourse.bass_utils import run_bass_kernel_spmd
    nc = _get_program(x.shape[1])
    in_maps = host_prepare(x, w_in, w_out, rms_w)
    res = run_bass_kernel_spmd(nc, in_maps, list(range(NCORES)), trace=trace)
    return assemble(x, res.results), res


def kernel(x, w_in, w_out, rms_w):
    out, _ = run(np.asarray(x), np.asarray(w_in), np.asarray(w_out),
                 np.asarray(rms_w))
    return out



# revision 11
# speedup vs baseline: 1.0014x; 1.0014x over previous
"""GroupedQueryAttention Trainium2 kernel (8-core SPMD), v6.

Reference op: RMSNorm -> in-proj (q/k/v) -> RoPE -> causal GQA attention
-> out-proj -> residual.  b=2, s=2048, d_model=2048, 32 q-heads / 8 KV
groups, head dim 64, fp32 in/out (f16 on device).

Sharding: core c handles batch b = c//4 and KV groups (2j, 2j+1), j = c%4.
Each core computes the in-projection restricted to its 8 heads' channels,
attention for its 8 heads, and a partial out-projection (row-parallel).
The host sums the 4 partials per batch and adds the residual.

v6 design (vs v2 baseline):
  * inv_rms precomputed on the host (a [B,S] reduction is negligible in
    numpy): removes all x^2 ops, 256 ss matmuls and the Newton chain.
  * causal mask folded into the PSUM scores on the PE itself: one
    id.T @ [mneg|mneg] accumulation per diagonal tile adds -6e4 above
    the diagonal for both heads, so exp yields exact zeros and no
    DVE mask sits between exp and the AV matmuls.
  * rope fully in f16 on DVE (2x mode) with single-replica cos/sin
    tables read through stride-0 mid-dim broadcast APs.
  * per-q-tile early softmax normalization: q-tile qt is normalized and
    queued for its o-pack transpose right after its last AV
    contribution (t = 4c+qt), shrinking the next pair's WAR stall on
    the shared PSUM accumulators.
  * x stored tok-tile-major in DRAM ([128, tt, kt, 128]) and loaded as
    4 per-tile DMAs; w_in preload split in kt-quarters ordered for the
    cold start; output DMA split per 4 m-slices for the tail.
  * tr-pool double buffered; qkT drains on DVE; emission weave as v2:
    P0(c) interleaves with ATT(c-1) at tok-tile granularity with
    transpose/o-pack fillers between matmul groups.
"""

import os
import numpy as np
from contextlib import ExitStack

import concourse.bass as bass
from concourse import bacc as _bacc
import concourse.mybir as mybir
import concourse.tile as tile
from concourse.bass import ts

f32 = mybir.dt.float32
f16 = mybir.dt.float16
AF = mybir.ActivationFunctionType
ALU = mybir.AluOpType

D = 2048          # model dim
CH = 768          # per-core in-proj channels (8 q heads + 2 k + 2 v)
TOKC = 512        # token chunk
NKT = D // 128    # 16 k-tiles over model dim
RMS_EPS = 1e-6
ROPE_THETA = 10000.0
NCORES = 8


def build_program(S=2048):
    NCH = S // TOKC          # token chunks
    NT = S // 128            # token/key tiles
    nc = _bacc.Bacc(None)

    xT_d = nc.dram_tensor("xT", [128, NT * NKT * 128], f16,
                          kind="ExternalInput")
    inv_d = nc.dram_tensor("invr", [128, NT], f32, kind="ExternalInput")
    w_in_d = nc.dram_tensor("w_in_p", [128, NKT * CH], f16, kind="ExternalInput")
    w_out_d = nc.dram_tensor("w_out_p", [128, 4 * D], f16, kind="ExternalInput")
    cos2_d = nc.dram_tensor("cos2", [128, NT * 64], f16, kind="ExternalInput")
    sinpm_d = nc.dram_tensor("sinpm", [128, NT * 64], f16, kind="ExternalInput")
    mneg_d = nc.dram_tensor("mneg", [128, 256], f16, kind="ExternalInput")
    id_d = nc.dram_tensor("id128", [128, 128], f16, kind="ExternalInput")
    yT_d = nc.dram_tensor("yT", [D, S], f16, kind="ExternalOutput")

    with tile.TileContext(nc) as tc, ExitStack() as ctx:
        sb = ctx.enter_context(tc.tile_pool(name="sb", bufs=1))
        sbs = ctx.enter_context(tc.tile_pool(name="sbs", bufs=2))

        # ---------------- persistent SBUF ----------------
        w_in_sb = sb.tile([128, NKT, CH], f16, name="w_in_sb")
        w_out_sb = sb.tile([128, 4, D], f16, name="w_out_sb")
        cos2_sb = sb.tile([128, NT, 64], f16, name="cos2_sb")
        sinpm_sb = sb.tile([128, NT, 64], f16, name="sinpm_sb")
        mneg_sb = sb.tile([128, 2, 128], f16, name="mneg_sb")
        id_sb = sb.tile([128, 128], f16, name="id_sb")
        ones_sb = sb.tile([128, 1], f16, name="ones_sb")
        zer_sb = sb.tile([128, 4, 65], f16, name="zer_sb")
        qkT = sb.tile([128, 5, S], f16, name="qkT")     # feat-major roped q(4)/k(1)
        vAB = sb.tile([128, NT, 2, 65], f16, name="vAB")
        oT = sb.tile([128, 4, S], f16, name="oT")       # feat-major o per pair
        inv_sb = sb.tile([128, NT], f32, name="inv_sb")  # host-computed rsqrt

        # preloads all on the scalar queue, ordered by first use; the sync
        # queue stays free so the x(0) load (emitted in the schedule) is
        # serviced immediately.
        w_in_r = w_in_d.rearrange("p (o c) -> p o c", c=CH)
        nc.scalar.dma_start(inv_sb[:], inv_d[:])
        nc.scalar.dma_start(w_in_sb[:, 0:4, :], w_in_r[:, 0:4, :])
        nc.scalar.dma_start(cos2_sb[:], cos2_d.rearrange("p (o c) -> p o c", c=64))
        nc.scalar.dma_start(sinpm_sb[:],
                            sinpm_d.rearrange("p (o c) -> p o c", c=64))
        for q in range(1, 4):
            nc.scalar.dma_start(w_in_sb[:, 4 * q:4 * q + 4, :],
                                w_in_r[:, 4 * q:4 * q + 4, :])
        nc.scalar.dma_start(mneg_sb[:],
                            mneg_d.rearrange("p (h i) -> p h i", i=128))
        nc.scalar.dma_start(id_sb[:], id_d[:])
        nc.scalar.dma_start(w_out_sb[:], w_out_d.rearrange("p (o c) -> p o c", c=D))
        nc.gpsimd.memset(ones_sb[:], 1.0)
        nc.gpsimd.memset(zer_sb[:], 0.0)
        # contiguous full-tile memset; V copies later overwrite cols 0:64
        # of each [*, t, h] slice, leaving column 64 as the ones column.
        nc.gpsimd.memset(vAB[:], 1.0)

        with tc.tile_pool(name="ps", bufs=1, space="PSUM") as ps:
            # PSUM budget (8 banks): big 2x2 + avA 1 + avB 1 + ss 1 + tr 1.

            # deferred-emission queue: thunks sprinkled between matmul
            # groups so single-buffered PSUM tags never stall the PE.
            filler_q = []

            def drain(n=1):
                for _ in range(n):
                    if filler_q:
                        filler_q.pop(0)()

            def drain_all():
                while filler_q:
                    filler_q.pop(0)()

            xchunks = {}

            def emit_load_x(c):
                # tile-major DRAM layout: per-tok-tile DMAs, 4KB runs.
                xc = sbs.tile([128, 4, NKT, 128], f16, tag="xc", bufs=2,
                              name=f"xc_{c}")
                xr = xT_d.rearrange("p (tt kt i) -> p tt kt i", kt=NKT, i=128)
                for t in range(4):
                    nc.sync.dma_start(xc[:, t], xr[:, 4 * c + t])
                xchunks[c] = xc

            def emit_P0_tau(c, t):
                """in-proj for tok-tile t of chunk c + rope chain
                (inv_rms is precomputed host-side).  Fillers drain
                between k-tile groups."""
                xc = xchunks[c]
                tg = 4 * c + t
                ip = ps.tile([128, 2, TOKC], f32, tag="big", bufs=2,
                             name=f"ip_{c}_{t}")
                for kt in range(NKT):
                    nc.tensor.matmul(ip[:, 0, 0:384], xc[:, t, kt, :],
                                     w_in_sb[:, kt, 0:384],
                                     start=(kt == 0), stop=(kt == NKT - 1))
                    nc.tensor.matmul(ip[:, 1, 0:384], xc[:, t, kt, :],
                                     w_in_sb[:, kt, 384:768],
                                     start=(kt == 0), stop=(kt == NKT - 1))
                    if kt % 5 == 4:
                        drain()
                # --- apply inv_rms to the whole q/k block in two ACT
                # copies (per-partition scale); rope then reads SBUF with
                # unscaled, preloaded tables.
                qn = sbs.tile([128, 640], f16, tag="qn", bufs=3,
                              name=f"qn_{c}_{t}")
                nc.scalar.activation(qn[:, 0:384], ip[:, 0, 0:384], AF.Copy,
                                     scale=inv_sb[:, tg:tg + 1])
                nc.scalar.activation(qn[:, 384:640], ip[:, 1, 0:256], AF.Copy,
                                     scale=inv_sb[:, tg:tg + 1])
                def bc(tab, lo, hi, h):
                    return tab[:, tg, lo:hi].unsqueeze(1).to_broadcast(
                        (128, h, hi - lo))
                # --- rope (token-major).  Block A: q heads 0-5; block B:
                # q heads 6,7 + k0,k1 (all rope identically).
                qt_sb = sbs.tile([128, 640], f16, tag="qt_sb", bufs=3,
                                 name=f"qt_{c}_{t}")
                tmpA = sbs.tile([128, 6, 64], f16, tag="tmpA", bufs=2,
                                name=f"tmpA_{c}_{t}")
                tmpB = sbs.tile([128, 4, 64], f16, tag="tmpB", bufs=2,
                                name=f"tmpB_{c}_{t}")
                cqA = sbs.tile([128, 6, 64], f16, tag="cqA", bufs=2,
                               name=f"cqA_{c}_{t}")
                cqB = sbs.tile([128, 4, 64], f16, tag="cqB", bufs=2,
                               name=f"cqB_{c}_{t}")
                blkA = qn[:, 0:384].rearrange("p (h d) -> p h d", d=64)
                blkB = qn[:, 384:640].rearrange("p (h d) -> p h d", d=64)
                nc.vector.tensor_tensor(
                    tmpA[:, :, 0:32], blkA[:, :, 32:64],
                    bc(sinpm_sb, 0, 32, 6), ALU.mult)
                nc.vector.tensor_tensor(
                    tmpA[:, :, 32:64], blkA[:, :, 0:32],
                    bc(sinpm_sb, 32, 64, 6), ALU.mult)
                nc.vector.tensor_tensor(cqA[:], blkA[:], bc(cos2_sb, 0, 64, 6),
                                        ALU.mult)
                nc.vector.tensor_tensor(
                    qt_sb[:, 0:384].rearrange("p (h d) -> p h d", d=64),
                    cqA[:], tmpA[:], ALU.add)
                nc.vector.tensor_tensor(
                    tmpB[:, :, 0:32], blkB[:, :, 32:64],
                    bc(sinpm_sb, 0, 32, 4), ALU.mult)
                nc.vector.tensor_tensor(
                    tmpB[:, :, 32:64], blkB[:, :, 0:32],
                    bc(sinpm_sb, 32, 64, 4), ALU.mult)
                nc.vector.tensor_tensor(cqB[:], blkB[:], bc(cos2_sb, 0, 64, 4),
                                        ALU.mult)
                nc.vector.tensor_tensor(
                    qt_sb[:, 384:640].rearrange("p (h d) -> p h d", d=64),
                    cqB[:], tmpB[:], ALU.add)
                # --- V: per-token inv scale during PSUM->SBUF copy
                nc.vector.tensor_tensor(
                    vAB[:, tg, :, 0:64],
                    ip[:, 1, 256:384].rearrange("p (h d) -> p h d", d=64),
                    inv_sb[:, tg:tg + 1].to_broadcast((128, 2, 64)), ALU.mult)
                # transpose q/k of this tok-tile to feature-major
                # (deferred: reads qt_sb, which the DVE/Pool chain above
                # still has to produce; spread over later mm groups).
                for ct in range(5):
                    filler_q.append(
                        lambda tg=tg, ct=ct, qt_sb=qt_sb: emit_tr(tg, ct, qt_sb))

            def emit_tr(tg, ct, qt_sb):
                trp = ps.tile([128, 128], f16, tag="tr", bufs=2,
                              name=f"tr_{tg}_{ct}")
                nc.tensor.transpose(trp[:], qt_sb[:, ts(ct, 128)], id_sb[:])
                nc.vector.tensor_copy(qkT[:, ct, ts(tg, 128)], trp[:])

            def emit_att_pair(c, p):
                """attention for q-chunk c, head-pair p (heads of groups
                g0,g1 at q ch-tile p; k ch-tile 4)."""
                n_t = 4 * (c + 1)
                # One PSUM bank supports a single accumulation group at a
                # time (2KB zero region), so the 4 concurrent per-q-tile
                # accumulators share a bank via explicit memset + pure
                # accumulation (start=False, skip_group_check).
                avA = ps.tile([128, 4, 65], f32, tag="avA", bufs=1,
                              name=f"avA_{c}_{p}")
                avB = ps.tile([128, 4, 65], f32, tag="avB", bufs=1,
                              name=f"avB_{c}_{p}")
                qks = []

                def emit_qk(t):
                    j0 = max(0, t - 4 * c) * 128
                    diag = t >= 4 * c
                    qk = ps.tile([128, 2, TOKC], f32, tag="big", bufs=2,
                                 name=f"qk_{c}_{p}_{t}")
                    for h in (0, 1):
                        nc.tensor.matmul(
                            qk[:, h, j0:], qkT[64 * h:64 * h + 64, 4, ts(t, 128)],
                            qkT[64 * h:64 * h + 64, p,
                                c * TOKC + j0:(c + 1) * TOKC],
                            start=True, stop=not diag)
                    if diag:
                        # fold the causal mask into the PSUM scores for
                        # both heads at once: += id.T @ [mneg|mneg] adds
                        # -6e4 above the diagonal; exp gives exact zeros.
                        nc.tensor.matmul(
                            qk[:, :, j0:j0 + 128], id_sb[:], mneg_sb[:],
                            start=False, stop=True, skip_group_check=True)
                    qks.append(qk)

                emit_qk(0)
                # o-pack fillers + av zero-init fill the WAR window before
                # qk(1) can reuse the previous pair's PSUM buffer.
                drain(3)
                nc.tensor.matmul(avA[:], id_sb[:], zer_sb[:],
                                 start=True, stop=True)
                nc.tensor.matmul(avB[:], id_sb[:], zer_sb[:],
                                 start=True, stop=True)
                for t in range(n_t):
                    if t + 1 < n_t:
                        emit_qk(t + 1)
                    j0 = max(0, t - 4 * c) * 128
                    qk = qks[t]
                    e = sbs.tile([128, 2, TOKC], f16, tag="e", bufs=4,
                                 name=f"e_{c}_{p}_{t}")
                    nc.scalar.activation(e[:, :, j0:], qk[:, :, j0:], AF.Exp)
                    if t < n_t - 2:
                        drain()
                    for qt in range(4):
                        if 4 * c + qt < t:
                            continue
                        nc.tensor.matmul(avA[:, qt, :],
                                         e[:, 0, ts(qt, 128)], vAB[:, t, 0, :],
                                         start=False, stop=False,
                                         skip_group_check=True)
                        nc.tensor.matmul(avB[:, qt, :],
                                         e[:, 1, ts(qt, 128)], vAB[:, t, 1, :],
                                         start=False, stop=False,
                                         skip_group_check=True)
                    # q-tile qt gets its last AV contribution at
                    # t = 4c + qt: normalize + queue its o-pack right
                    # away so the next pair's WAR on av shrinks.
                    qt = t - 4 * c
                    if qt >= 0:
                        d2 = sbs.tile([128, 2], f32, tag="d2", bufs=4,
                                      name=f"d2_{c}_{p}_{qt}")
                        nc.vector.reciprocal(d2[:, 0:1], avA[:, qt, 64:65])
                        nc.vector.reciprocal(d2[:, 1:2], avB[:, qt, 64:65])
                        pk = sbs.tile([128, 128], f16, tag="pk", bufs=3,
                                      name=f"pk_{c}_{p}_{qt}")
                        nc.vector.tensor_tensor(
                            pk[:, 0:64], avA[:, qt, 0:64],
                            d2[:, 0:1].to_broadcast((128, 64)), ALU.mult)
                        nc.vector.tensor_tensor(
                            pk[:, 64:128], avB[:, qt, 0:64],
                            d2[:, 1:2].to_broadcast((128, 64)), ALU.mult)
                        filler_q.append(
                            lambda c=c, p=p, qt=qt, pk=pk:
                            emit_opack(c, p, qt, pk))

            def emit_opack(c, p, qt, pk):
                trp = ps.tile([128, 128], f16, tag="tr", bufs=2,
                              name=f"otr_{c}_{p}_{qt}")
                nc.tensor.transpose(trp[:], pk[:], id_sb[:])
                nc.vector.tensor_copy(oT[:, p, c * TOKC + qt * 128:
                                         c * TOKC + (qt + 1) * 128], trp[:])

            def emit_out(c):
                cs = slice(c * TOKC, (c + 1) * TOKC)
                yo = sbs.tile([128, 16, TOKC], f16, tag="yo", bufs=1,
                              name=f"yo_{c}")
                for m in range(16):
                    op = ps.tile([128, 2, TOKC], f32, tag="big", bufs=2,
                                 name=f"op_{c}_{m}")
                    for kt in range(4):
                        nc.tensor.matmul(op[:, 0, :], w_out_sb[:, kt, ts(m, 128)],
                                         oT[:, kt, cs],
                                         start=(kt == 0), stop=(kt == 3))
                    nc.vector.tensor_copy(yo[:, m, :], op[:, 0, :])
                    if m % 3 == 2:
                        drain()
                    if m % 4 == 3:
                        nc.sync.dma_start(
                            yT_d.rearrange("(o p) s -> p o s", p=128)[
                                0:128, m - 3:m + 1, cs], yo[:, m - 3:m + 1, :])

            # ------------------- schedule -------------------
            # depth-1 stagger: attention for chunk c-1 weaves into P0(c).
            # The drain_all() at iteration start flushes TR(c-1) so the
            # whole qkT range for keys <= c-1 is emitted before ATT(c-1).
            emit_load_x(0)
            for c in range(NCH + 1):
                drain_all()
                for t in range(4):
                    if t == 2 and c + 1 < NCH:
                        emit_load_x(c + 1)
                    if c < NCH:
                        emit_P0_tau(c, t)
                    if 0 <= c - 1 < NCH:
                        emit_att_pair(c - 1, t)
                if 0 <= c - 1 < NCH:
                    drain_all()   # flush o-pack of pair 3 before out-proj
                    emit_out(c - 1)
            drain_all()

    nc.finalize()
    return nc


# ------------------------------- host side ----------------------------------

def _rope_tables(S):
    NT = S // 128
    inv_freq = ROPE_THETA ** (-np.arange(0, 64, 2, dtype=np.float64) / 64.0)
    t = np.arange(S, dtype=np.float64)[:, None]            # [S, 1]
    ang = t * inv_freq[None, :]                            # [S, 32]
    cos = np.cos(ang)
    sin = np.sin(ang)
    cos2 = np.concatenate([cos, cos], axis=1)              # [S, 64]
    sinpm = np.concatenate([-sin, sin], axis=1)            # [S, 64]
    cos2 = cos2.reshape(NT, 128, 64).transpose(1, 0, 2).reshape(128, NT * 64)
    sinpm = sinpm.reshape(NT, 128, 64).transpose(1, 0, 2).reshape(128, NT * 64)
    return (np.ascontiguousarray(cos2, dtype=np.float16),
            np.ascontiguousarray(sinpm, dtype=np.float16))


def host_prepare(x, w_in, w_out, rms_w):
    S = x.shape[1]
    NT = S // 128
    x = np.asarray(x, dtype=np.float32)
    w_eff = np.asarray(w_in, dtype=np.float32) * np.asarray(rms_w, np.float32)[None, :]
    w_out = np.asarray(w_out, dtype=np.float32)
    cos2, sinpm = _rope_tables(S)
    mneg1 = np.tril(np.full((128, 128), -60000.0, dtype=np.float32), -1)
    mneg = np.ascontiguousarray(np.concatenate([mneg1, mneg1], axis=1))
    id128 = np.eye(128, dtype=np.float32)
    qscale = np.float32(64 ** -0.5)
    # host-side RMS inverse: [B, S] -> per-core [128, NT]
    inv_all = 1.0 / np.sqrt((x.astype(np.float32) ** 2).mean(-1) + RMS_EPS)

    in_maps = []
    for core in range(NCORES):
        b, j = divmod(core, 4)
        g0, g1 = 2 * j, 2 * j + 1
        rows = []
        for p in range(4):
            for g in (g0, g1):
                rows.extend(range((g * 4 + p) * 64, (g * 4 + p) * 64 + 64))
        for g in (g0, g1):
            rows.extend(range(2048 + g * 64, 2048 + g * 64 + 64))
        for g in (g0, g1):
            rows.extend(range(2560 + g * 64, 2560 + g * 64 + 64))
        w_slice = w_eff[rows, :].copy()          # [768, 2048]
        w_slice[:512, :] *= qscale
        # device layout: w_in_p[p, kt*768 + ch] = w_slice[ch, kt*128 + p]
        w_in_p = w_slice.T.reshape(NKT, 128, CH).transpose(1, 0, 2).reshape(
            128, NKT * CH)
        cols = []
        for p in range(4):
            for g in (g0, g1):
                cols.extend(range((g * 4 + p) * 64, (g * 4 + p) * 64 + 64))
        w_o = w_out[:, cols]                     # [2048, 512]
        # device layout: w_out_p[p, kt*2048 + m] = w_o[m, kt*128 + p]
        w_out_p = w_o.T.reshape(4, 128, D).transpose(1, 0, 2).reshape(128, 4 * D)
        xb = x[b].reshape(NT, 128, NKT, 128)     # [tt, i, kt, p]
        xT = xb.transpose(3, 0, 2, 1).reshape(128, NT * NKT * 128)
        invr = np.ascontiguousarray(
            inv_all[b].reshape(NT, 128).T).astype(np.float32)
        in_maps.append({
            "xT": np.ascontiguousarray(xT).astype(np.float16),
            "invr": invr,
            "w_in_p": np.ascontiguousarray(w_in_p).astype(np.float16),
            "w_out_p": np.ascontiguousarray(w_out_p).astype(np.float16),
            "cos2": cos2.astype(np.float16),
            "sinpm": sinpm.astype(np.float16),
            "mneg": mneg.astype(np.float16),
            "id128": id128.astype(np.float16),
        })
    return in_maps


def assemble(x, results):
    x = np.asarray(x, dtype=np.float32)
    out = np.empty_like(x)
    for b in range(2):
        acc = np.zeros((D, x.shape[1]), dtype=np.float32)
        for j in range(4):
            acc += results[4 * b + j]["yT"].astype(np.float32)
        out[b] = x[b] + acc.T
    return out


_PROGRAMS = {}


def _get_program(S):
    if S not in _PROGRAMS:
        _PROGRAMS[S] = build_program(S)
    return _PROGRAMS[S]


def run(x, w_in, w_out, rms_w, trace=False):
    from concourse.bass_utils import run_bass_kernel_spmd
    nc = _get_program(x.shape[1])
    in_maps = host_prepare(x, w_in, w_out, rms_w)
    res = run_bass_kernel_spmd(nc, in_maps, list(range(NCORES)), trace=trace)
    return assemble(x, res.results), res


def kernel(x, w_in, w_out, rms_w):
    out, _ = run(np.asarray(x), np.asarray(w_in), np.asarray(w_out),
                 np.asarray(rms_w))
    return out

